# revision 20
# baseline (speedup 1.0000x reference)
"""Distributed Bass attention kernel for 8 TRN2 NeuronCores.

Problem: full-dim attention (no head split), x:(2,4096,2048), 4x 2048^2 weights.

Sharding: batch+sequence parallel. Core c owns batch b=c//4 and query rows
[1024*(c%4), 1024*(c%4+1)).

Algebraic restructure vs the classic q/k/v/o pipeline (all bf16; fp8 was
measured numerically unusable here -- heavy-tailed scores make sharp softmax
rows pass element-level e4m3 noise straight to the output):

- chained QK: scores = ((x Wq^T) Wk) x_full^T. The second projection
  q' = q @ Wk replaces the k-projection; x_full^T is fed from the host, so
  there is NO AllGather(K) at all and the score phase has no collective
  dependency.
- fused VO: W_vo^T = Wv^T Wo^T is computed on-chip, 4-way sharded within
  each replica group (each core computes 512 rows; one AllGather within the
  group, fully hidden behind the q/q' projections). Then v' = x @ W_vo^T is
  gathered (AG per 512-col quarter, hidden behind v'-proj+scores) and
  ctx = softmax(scores) @ v' IS the final output -- no output projection.
  Net: -512 weight-column matmuls +256 slice matmuls ~= -67us of PE rows.

Schedule notes:
- Scores are computed TRANSPOSED (P^T[j,i] via lhsT=x_full^T chunk) so exp
  writes P^T directly; no max subtraction (validated numerically, rel err
  ~5e-3 vs 2e-2 budget).
- ctx is computed UNtransposed (out[i-part, a] via lhsT=P^T tiles), so the
  store to out[i, a] needs no transpose; 1/l is applied during the PSUM
  evacuation as a per-partition activation scale.
- Row sums l[i] need i on partitions -> 256 tiny FD-1 matmuls
  (lhsT = P^T tile, rhs = ones column), interleaved per score chunk so no
  serial rowsum stage exists.
- The PE is GPIO-power-throttled to 13/16 rate (~1.95 rows/ns) for ~95% of
  the kernel; wall time ~= matmul rows x 0.52ns + exposed latency. The
  schedule keeps AG(Wvo) behind the q/q' projections and AG(v') quarters
  behind v'-proj+scores so no collective is on the PE's critical path.
- DMA rings: scalar = weight-column streams then the x_full^T score
  chunks; sync = slice weights, x^T, gathered-Wvo quarters, output stores;
  gpsimd = v' bounce stores and ctx V-column loads.
"""

import numpy as np
import ml_dtypes

BF16 = ml_dtypes.bfloat16

D = 2048          # model dim
S = 4096          # sequence length per batch
BATCH = 2
NCORES = 8
GROUP = 4         # replica group size (cores per batch)
ROWS = S // GROUP  # query rows per core = 1024
P = 128           # partitions
DT = D // P       # 16 d-tiles
ET = DT
BT = DT
IT = ROWS // P    # 8 i-tiles per core
JT = S // P       # 32 j-tiles (full seq)
NCH = 16          # score chunks of 256 keys
SCALE = 1.0 / float(np.sqrt(D))

_CACHE = {}


def _build():
    from concourse import bacc, mybir, tile

    f32 = mybir.dt.float32
    bf16 = mybir.dt.bfloat16

    nc = bacc.Bacc("TRN2", target_bir_lowering=False, debug=False,
                   num_devices=NCORES)

    # host-pre-tiled inputs (see _in_maps): every load is contiguous rows
    xt_d = nc.dram_tensor("xt", [P, DT * 1024], bf16, kind="ExternalInput")
    xk_d = nc.dram_tensor("xk", [NCH, P, DT * 256], bf16,
                          kind="ExternalInput")
    wqt_d = nc.dram_tensor("wqt", [ET, P, DT * P], bf16, kind="ExternalInput")
    wkt_d = nc.dram_tensor("wkt", [DT, P, ET * P], bf16, kind="ExternalInput")
    wvs_d = nc.dram_tensor("wvs", [P, BT * 512], bf16, kind="ExternalInput")
    wos_d = nc.dram_tensor("wos", [4, P, BT * 512], bf16,
                           kind="ExternalInput")
    out_d = nc.dram_tensor("out", [ROWS, D], f32, kind="ExternalOutput")

    RG4 = [[0, 1, 2, 3], [4, 5, 6, 7]]

    def all_gather(src, dst):
        return nc.gpsimd.collective_compute(
            "AllGather", mybir.AluOpType.bypass, replica_groups=RG4,
            ins=[src.opt()], outs=[dst.opt()])

    with tile.TileContext(nc) as tc:
        dram = tc.alloc_tile_pool(name="dram", bufs=1, space="DRAM")
        persist = tc.alloc_tile_pool(name="persist", bufs=1)
        psum = tc.alloc_tile_pool(name="psum", bufs=2, space="PSUM")

        # W_vo^T slice bounce + gather; v' bounce/gather per 512-col quarter
        wvo_b = dram.tile([4 * P, D], bf16, name="wvo_b")
        wvo_g = dram.tile([GROUP, 4 * P, D], bf16, name="wvo_g")
        v_b = [dram.tile([ROWS, 512], bf16, name=f"v_b{h}") for h in range(4)]
        v_g = [dram.tile([GROUP, ROWS, 512], bf16, name=f"v_g{h}")
               for h in range(4)]
        l_d = dram.tile([IT, P], f32, name="l_d")

        ones = persist.tile([P, P], bf16)
        linv = persist.tile([P, IT], f32)  # 1/l, i on partitions

        # q'^T: written in phase 2, read through the score phase; sits at
        # the bottom of the left stack so the LIFO release order works out
        pq2 = tc.alloc_tile_pool(name="pq2", bufs=1)
        qpt = pq2.tile([P, DT, 1024], bf16)

        # x^T: needed by q-proj (~t=75) and v'-proj; own pool, lives
        # through v'-proj
        pxt = tc.alloc_tile_pool(name="pxt", bufs=1)
        xt_s = pxt.tile([P, DT, 1024], bf16)

        # weight-column stream pool sits BELOW pslice so the q-proj
        # columns preload during the slice phase with no WAR gate
        pw = tc.alloc_tile_pool(name="pw", bufs=2)

        # ---------------- Phase 1: W_vo^T slice ----------------
        pslice = tc.alloc_tile_pool(name="pslice", bufs=2)

        # warm both HWDGE rings so the first real loads skip spin-up
        warm = pslice.tile([P, 16], bf16, bufs=1)
        nc.sync.dma_start(out=warm[0:1, :], in_=xt_d[0:1, 0:16])
        nc.scalar.dma_start(out=warm[1:2, :], in_=xt_d[1:2, 0:16])
        nc.gpsimd.memset(ones[:], 1.0)

        # slice lhsT (Wv columns for this core's 512 W_vo rows) in 3 chunks,
        # then the rhs quarters stream; first matmul gates on ~1.5MB
        wvs_s = pslice.tile([P, BT, 512], bf16, bufs=1)
        nc.scalar.dma_start(out=wvs_s[:, 0:4, :], in_=wvs_d[:, :4 * 512])
        nc.scalar.dma_start(out=wvs_s[:, 4:8, :],
                            in_=wvs_d[:, 4 * 512:8 * 512])
        nc.scalar.dma_start(out=wvs_s[:, 8:16, :], in_=wvs_d[:, 8 * 512:])
        wos_t = []
        for ab in range(4):
            w = pslice.tile([P, BT, 512], bf16, tag="wos", bufs=2)
            eng = nc.sync if ab % 2 == 0 else nc.scalar
            for g in range(4):
                eng.dma_start(out=w[:, 4 * g:4 * g + 4, :],
                              in_=wos_d[ab][:, 4 * g * 512:(4 * g + 4) * 512])
            wos_t.append(w)

        # x^T loads behind the first slice weights on sync
        for g in range(4):
            nc.sync.dma_start(out=xt_s[:, 4 * g:4 * g + 4, :],
                              in_=xt_d[:, 4 * g * 1024:(4 * g + 4) * 1024])

        # W_vo^T[c-slice, a] = sum_b Wv[b, c-slice]^T Wo^T[b, a]
        stage = pslice.tile([P, 4, D], bf16, bufs=1)
        for ab in range(4):
            wos_ab = wos_t[ab]
            for ct in range(4):
                ps = psum.tile([P, 512], f32, tag="acc")
                for bt in range(BT):
                    nc.tensor.matmul(
                        ps[:],
                        wvs_s[:, bt, ct * P:(ct + 1) * P],
                        wos_ab[:, bt, :],
                        start=(bt == 0),
                        stop=(bt == BT - 1))
                nc.vector.tensor_copy(
                    stage[:, ct, ab * 512:(ab + 1) * 512], ps[:])
        for ct in range(4):
            (nc.sync if ct % 2 == 0 else nc.scalar).dma_start(
                out=wvo_b[ct * P:(ct + 1) * P, :], in_=stage[:, ct, :])
        all_gather(wvo_b, wvo_g)

        # gathered W_vo^T quarters (a-block each) on the RIGHT side; the
        # sync ring is otherwise idle from here until the output stores,
        # so the AG-gated loads block nothing
        wvopool = tc.alloc_tile_pool(name="wvopool", bufs=1, side="right")
        wvoq = []
        for ab in range(4):
            wq_t = wvopool.tile([P, DT, 512], bf16, tag="wvoq", bufs=2)
            for gct in range(DT):
                r, ctl = gct // 4, gct % 4
                nc.sync.dma_start(
                    out=wq_t[:, gct, :],
                    in_=wvo_g[r, ctl * P:(ctl + 1) * P,
                              ab * 512:(ab + 1) * 512])
            wvoq.append(wq_t)
        pslice.release()

        # ---------------- Phase 2: q then q' = q @ Wk ----------------
        pq = tc.alloc_tile_pool(name="pq", bufs=1)
        qt_s = pq.tile([P, ET, 1024], bf16)
        for et in range(ET):
            wcol = pw.tile([P, DT, P], bf16, tag="wcol", bufs=6)
            nc.scalar.dma_start(out=wcol[:], in_=wqt_d[et])
            for ih in range(2):
                ps = psum.tile([P, 512], f32, tag="acc")
                for dt_i in range(DT):
                    nc.tensor.matmul(
                        ps[:],
                        wcol[:, dt_i, :],
                        xt_s[:, dt_i, ih * 512:(ih + 1) * 512],
                        start=(dt_i == 0),
                        stop=(dt_i == DT - 1))
                nc.vector.tensor_copy(
                    qt_s[:, et, ih * 512:(ih + 1) * 512], ps[:])

        for d2t in range(DT):
            wcol = pw.tile([P, ET, P], bf16, tag="wcol", bufs=6)
            nc.scalar.dma_start(out=wcol[:], in_=wkt_d[d2t])
            for ih in range(2):
                ps = psum.tile([P, 512], f32, tag="acc")
                for et in range(ET):
                    nc.tensor.matmul(
                        ps[:],
                        wcol[:, et, :],
                        qt_s[:, et, ih * 512:(ih + 1) * 512],
                        start=(et == 0),
                        stop=(et == ET - 1))
                nc.vector.tensor_copy(
                    qpt[:, d2t, ih * 512:(ih + 1) * 512], ps[:])
        pq.release()
        pw.release()

        # ---------------- Phase 3: v' = x @ W_vo^T ----------------
        pv = tc.alloc_tile_pool(name="pv", bufs=2)
        for ab in range(4):
            v_st = pv.tile([P, IT, 512], bf16, tag="v_st", bufs=2)
            for jt in range(IT):
                ps = psum.tile([P, 512], f32, tag="acc")
                for ct in range(DT):
                    nc.tensor.matmul(
                        ps[:],
                        xt_s[:, ct, jt * P:(jt + 1) * P],
                        wvoq[ab][:, ct, :],
                        start=(ct == 0),
                        stop=(ct == DT - 1))
                nc.vector.tensor_copy(v_st[:, jt, :], ps[:])
            nc.gpsimd.dma_start(
                out=v_b[ab][:].rearrange("(jt p) d -> p jt d", p=P),
                in_=v_st[:])
            all_gather(v_b[ab], v_g[ab])
        pv.release()
        wvopool.release()
        pxt.release()

        # ---------------- Phase 4: scores + rowsums ----------------
        ppt = tc.alloc_tile_pool(name="ppt", bufs=1, side="right")
        pt_s = ppt.tile([P, JT, 1024], bf16)
        att = tc.alloc_tile_pool(name="att", bufs=2)
        # first ctx V-column tiles prefetch on gpsimd as soon as AG(v'0)
        # lands, long before the ctx phase. vcol slot rotation (4 tiles per
        # quarter, 6 slots) leaves each quarter's first two tiles on fresh
        # slots, so the identity consumption order never waits on a WAR.
        oct_pool = tc.alloc_tile_pool(name="oct", bufs=2, side="right")

        def vcol_load(ab):
            vcols = [oct_pool.tile([P, IT, 512], bf16, tag="vcol",
                                   bufs=6, name=f"vcol{ab}_{r}")
                     for r in range(GROUP)]
            for r in range(GROUP):
                nc.gpsimd.dma_start(
                    out=vcols[r][:],
                    in_=v_g[ab][r, :, :].rearrange("(t p) d -> p t d", p=P))
            return vcols

        vcol0 = vcol_load(0)

        for ch in range(NCH):
            xkb = att.tile([P, DT, 256], bf16, tag="xkb", bufs=3)
            eng = nc.sync if ch % 2 == 0 else nc.scalar
            for g in range(2):
                eng.dma_start(
                    out=xkb[:, 8 * g:8 * g + 8, :],
                    in_=xk_d[ch][:, 8 * g * 256:(8 * g + 8) * 256])
            for jl in range(2):
                jt = ch * 2 + jl
                for ib in range(2):
                    sps = psum.tile([P, 512], f32, tag="scores", bufs=4)
                    for dt_i in range(DT):
                        nc.tensor.matmul(
                            sps[:],
                            xkb[:, dt_i, jl * P:(jl + 1) * P],
                            qpt[:, dt_i, ib * 512:(ib + 1) * 512],
                            start=(dt_i == 0),
                            stop=(dt_i == DT - 1))
                    nc.scalar.activation(
                        pt_s[:, jt, ib * 512:(ib + 1) * 512],
                        sps[:],
                        mybir.ActivationFunctionType.Exp,
                        scale=SCALE)
        # row sums via broadcast ones-matmul (l replicated on all
        # partitions), then a 4KB DRAM bounce transposes l onto
        # i-partitions for the ctx evacuation scale. The bounce chain
        # hides behind the first ctx accumulation group.
        l_sb = att.tile([P, 2, 512], f32, bufs=1)
        for ib in range(2):
            lps = psum.tile([P, 512], f32, tag="scores", bufs=4)
            for jt in range(JT):
                nc.tensor.matmul(
                    lps[:], ones[:],
                    pt_s[:, jt, ib * 512:(ib + 1) * 512],
                    start=(jt == 0), stop=(jt == JT - 1))
            nc.vector.tensor_copy(l_sb[:, ib, :], lps[:])
        nc.sync.dma_start(out=l_d[:, :], in_=l_sb[0:1, :, :])
        l_t = att.tile([P, IT], f32, bufs=1)
        for it in range(IT):
            nc.sync.dma_start(out=l_t[:, it:it + 1], in_=l_d[it:it + 1, :])
        nc.vector.reciprocal(linv[:], l_t[:])
        att.release()
        pq2.release()

        # ---------------- Phase 5: ctx = P^T^T @ v' = output ----------
        for ab in range(4):
            vcols = vcol0 if ab == 0 else vcol_load(ab)
            for h in range(2):
                osb = oct_pool.tile([P, 4, 512], f32, tag="osb", bufs=2)
                for il in range(4):
                    it = h * 4 + il
                    cps = psum.tile([P, 512], f32, tag="scores", bufs=4)
                    for jt in range(JT):
                        nc.tensor.matmul(
                            cps[:],
                            pt_s[:, jt, it * P:(it + 1) * P],
                            vcols[jt // IT][:, jt % IT, :],
                            start=(jt == 0),
                            stop=(jt == JT - 1))
                    nc.scalar.activation(
                        osb[:, il, :], cps[:],
                        mybir.ActivationFunctionType.Copy,
                        scale=linv[:, it:it + 1])
                # merged stores; final block split so the post-last-matmul
                # tail is a short store
                last = ab == 3 and h == 1
                bounds = ((0, 4),) if not last else ((0, 2), (2, 3), (3, 4))
                for lo, hi in bounds:
                    nc.sync.dma_start(
                        out=out_d[(4 * h + lo) * P:(4 * h + hi) * P,
                                  ab * 512:(ab + 1) * 512]
                        .rearrange("(it p) f -> p it f", p=P),
                        in_=osb[:, lo:hi, :])
        oct_pool.release()
        ppt.release()
        persist.release()
        psum.release()
        dram.release()

    nc.compile()
    return nc


def _get_nc():
    if "nc" not in _CACHE:
        _CACHE["nc"] = _build()
    return _CACHE["nc"]


def _tile_we(w):
    # lhsT matrix L = w.T tiled as [mt, p, pt*128] contiguous
    wt = np.ascontiguousarray(np.asarray(w, np.float32).T)
    t = wt.reshape(DT, P, DT, P).transpose(2, 1, 0, 3)
    return np.ascontiguousarray(t.reshape(DT, P, DT * P)).astype(BF16)


def _in_maps(x, wq, wk, wv, wo):
    x = np.asarray(x, np.float32)
    wq = np.asarray(wq, np.float32)
    wk = np.asarray(wk, np.float32)
    wv = np.asarray(wv, np.float32)
    wo = np.asarray(wo, np.float32)

    wqt = _tile_we(wq)            # q-proj lhsT: Wq^T tiles [et, p=d, dt, e]
    wkt = _tile_we(wk.T)          # q'-proj lhsT: Wk tiles [d2t, p=e, et, d2]
    # rhs Wo^T[b, a] tiled [ab, p=b, bt, 512]
    wot = wo.T.reshape(BT, P, 4, 512).transpose(2, 1, 0, 3)
    wot = np.ascontiguousarray(wot.reshape(4, P, BT * 512)).astype(BF16)

    maps = []
    for c in range(NCORES):
        b, r = c // GROUP, c % GROUP
        xb = x[b]                                     # [4096, 2048]
        xl = xb[r * ROWS:(r + 1) * ROWS, :]           # [1024, 2048]
        xt = xl.T.reshape(DT, P, ROWS).transpose(1, 0, 2)
        xt = np.ascontiguousarray(xt.reshape(P, DT * ROWS)).astype(BF16)
        # x_full^T chunks [ch, p=d, dt, 256]
        xk = xb.T.reshape(DT, P, NCH, 256).transpose(2, 1, 0, 3)
        xk = np.ascontiguousarray(xk.reshape(NCH, P, DT * 256)).astype(BF16)
        # slice lhsT: Wv[:, 512-col slice] tiled [p=b, bt, 512]
        wvs = wv[:, r * 512:(r + 1) * 512].reshape(BT, P, 512)
        wvs = np.ascontiguousarray(
            wvs.transpose(1, 0, 2).reshape(P, BT * 512)).astype(BF16)
        maps.append({"xt": xt, "xk": xk, "wqt": wqt, "wkt": wkt,
                     "wvs": wvs, "wos": wot})
    return maps


def run(x, wq, wk, wv, wo, trace=False, **trace_kwargs):
    from concourse.bass_utils import run_bass_kernel_spmd
    nc = _get_nc()
    res = run_bass_kernel_spmd(nc, _in_maps(x, wq, wk, wv, wo),
                               list(range(NCORES)), trace=trace,
                               **trace_kwargs)
    out = np.empty((BATCH, S, D), np.float32)
    for c in range(NCORES):
        b, r = c // GROUP, c % GROUP
        out[b, r * ROWS:(r + 1) * ROWS, :] = res.results[c]["out"]
    return out, res


def kernel(x, wq, wk, wv, wo):
    out, _ = run(x, wq, wk, wv, wo)
    return out


# revision 22
# speedup vs baseline: 1.0061x; 1.0061x over previous
"""Distributed Bass attention kernel for 8 TRN2 NeuronCores.

Problem: full-dim attention (no head split), x:(2,4096,2048), 4x 2048^2 weights.

Sharding: batch+sequence parallel. Core c owns batch b=c//4 and query rows
[1024*(c%4), 1024*(c%4+1)).

Algebraic restructure vs the classic q/k/v/o pipeline (all bf16; fp8 was
measured numerically unusable here -- heavy-tailed scores make sharp softmax
rows pass element-level e4m3 noise straight to the output):

- chained QK: scores = ((x Wq^T) Wk) x_full^T. The second projection
  q' = q @ Wk replaces the k-projection; x_full^T is fed from the host, so
  there is NO AllGather(K) at all and the score phase has no collective
  dependency.
- fused VO: W_vo^T = Wv^T Wo^T is computed on-chip, 4-way sharded within
  each replica group (each core computes 512 rows; one AllGather within the
  group, fully hidden behind the q/q' projections). Then v' = x @ W_vo^T is
  gathered (AG per 512-col quarter, hidden behind v'-proj+scores) and
  ctx = softmax(scores) @ v' IS the final output -- no output projection.
  Net: -512 weight-column matmuls +256 slice matmuls ~= -67us of PE rows.

Schedule notes:
- Scores are computed TRANSPOSED (P^T[j,i] via lhsT=x_full^T chunk) so exp
  writes P^T directly; no max subtraction (validated numerically, rel err
  ~5e-3 vs 2e-2 budget).
- ctx is computed UNtransposed (out[i-part, a] via lhsT=P^T tiles), so the
  store to out[i, a] needs no transpose; 1/l is applied during the PSUM
  evacuation as a per-partition activation scale.
- Row sums l[i] need i on partitions -> 256 tiny FD-1 matmuls
  (lhsT = P^T tile, rhs = ones column), interleaved per score chunk so no
  serial rowsum stage exists.
- The PE is GPIO-power-throttled to 13/16 rate (~1.95 rows/ns) for ~95% of
  the kernel; wall time ~= matmul rows x 0.52ns + exposed latency. The
  schedule keeps AG(Wvo) behind the q/q' projections and AG(v') quarters
  behind v'-proj+scores so no collective is on the PE's critical path.
- DMA rings: scalar = weight-column streams then the x_full^T score
  chunks; sync = slice weights, x^T, gathered-Wvo quarters, output stores;
  gpsimd = v' bounce stores and ctx V-column loads.
"""

import numpy as np
import ml_dtypes

BF16 = ml_dtypes.bfloat16

D = 2048          # model dim
S = 4096          # sequence length per batch
BATCH = 2
NCORES = 8
GROUP = 4         # replica group size (cores per batch)
ROWS = S // GROUP  # query rows per core = 1024
P = 128           # partitions
DT = D // P       # 16 d-tiles
ET = DT
BT = DT
IT = ROWS // P    # 8 i-tiles per core
JT = S // P       # 32 j-tiles (full seq)
NCH = 16          # score chunks of 256 keys
SCALE = 1.0 / float(np.sqrt(D))

_CACHE = {}


def _build():
    from concourse import bacc, mybir, tile

    f32 = mybir.dt.float32
    bf16 = mybir.dt.bfloat16

    nc = bacc.Bacc("TRN2", target_bir_lowering=False, debug=False,
                   num_devices=NCORES)

    # host-pre-tiled inputs (see _in_maps): every load is contiguous rows
    xt_d = nc.dram_tensor("xt", [P, DT * 1024], bf16, kind="ExternalInput")
    xk_d = nc.dram_tensor("xk", [NCH, P, DT * 256], bf16,
                          kind="ExternalInput")
    wqt_d = nc.dram_tensor("wqt", [ET, P, DT * P], bf16, kind="ExternalInput")
    wkt_d = nc.dram_tensor("wkt", [DT, P, ET * P], bf16, kind="ExternalInput")
    wvs_d = nc.dram_tensor("wvs", [P, BT * 512], bf16, kind="ExternalInput")
    wos_d = nc.dram_tensor("wos", [4, P, BT * 512], bf16,
                           kind="ExternalInput")
    out_d = nc.dram_tensor("out", [ROWS, D], f32, kind="ExternalOutput")

    RG4 = [[0, 1, 2, 3], [4, 5, 6, 7]]

    def all_gather(src, dst):
        return nc.gpsimd.collective_compute(
            "AllGather", mybir.AluOpType.bypass, replica_groups=RG4,
            ins=[src.opt()], outs=[dst.opt()])

    with tile.TileContext(nc) as tc:
        dram = tc.alloc_tile_pool(name="dram", bufs=1, space="DRAM")
        persist = tc.alloc_tile_pool(name="persist", bufs=1)
        psum = tc.alloc_tile_pool(name="psum", bufs=2, space="PSUM")

        # W_vo^T slice bounce + gather; v' bounce/gather per 512-col quarter
        wvo_b = dram.tile([4, 4, P, 512], bf16, name="wvo_b")
        wvo_g = dram.tile([GROUP, 4, 4, P, 512], bf16,
                          name="wvo_g")
        v_b = [dram.tile([ROWS, 512], bf16, name=f"v_b{h}") for h in range(4)]
        v_g = [dram.tile([GROUP, ROWS, 512], bf16, name=f"v_g{h}")
               for h in range(4)]
        l_d = dram.tile([IT, P], f32, name="l_d")

        ones = persist.tile([P, P], bf16)
        linv = persist.tile([P, IT], f32)  # 1/l, i on partitions

        # q'^T: written in phase 2, read through the score phase; sits at
        # the bottom of the left stack so the LIFO release order works out
        pq2 = tc.alloc_tile_pool(name="pq2", bufs=1)
        qpt = pq2.tile([P, DT, 1024], bf16)

        # x^T: needed by q-proj (~t=75) and v'-proj; own pool, lives
        # through v'-proj
        pxt = tc.alloc_tile_pool(name="pxt", bufs=1)
        xt_s = pxt.tile([P, DT, 1024], bf16)

        # weight-column stream pool sits BELOW pslice so the q-proj
        # columns preload during the slice phase with no WAR gate
        pw = tc.alloc_tile_pool(name="pw", bufs=2)

        # ---------------- Phase 1: W_vo^T slice ----------------
        pslice = tc.alloc_tile_pool(name="pslice", bufs=2)

        # warm both HWDGE rings so the first real loads skip spin-up
        warm = pslice.tile([P, 16], bf16, bufs=1)
        nc.sync.dma_start(out=warm[0:1, :], in_=xt_d[0:1, 0:16])
        nc.scalar.dma_start(out=warm[1:2, :], in_=xt_d[1:2, 0:16])
        nc.gpsimd.memset(ones[:], 1.0)

        # slice lhsT (Wv columns for this core's 512 W_vo rows) in 3 chunks,
        # then the rhs quarters stream; first matmul gates on ~1.5MB
        wvs_s = pslice.tile([P, BT, 512], bf16, bufs=1)
        nc.scalar.dma_start(out=wvs_s[:, 0:4, :], in_=wvs_d[:, :4 * 512])
        nc.scalar.dma_start(out=wvs_s[:, 4:8, :],
                            in_=wvs_d[:, 4 * 512:8 * 512])
        nc.scalar.dma_start(out=wvs_s[:, 8:16, :], in_=wvs_d[:, 8 * 512:])
        wos_t = []
        for ab in range(4):
            w = pslice.tile([P, BT, 512], bf16, tag="wos", bufs=2)
            eng = nc.sync if ab % 2 == 0 else nc.scalar
            for g in range(4):
                eng.dma_start(out=w[:, 4 * g:4 * g + 4, :],
                              in_=wos_d[ab][:, 4 * g * 512:(4 * g + 4) * 512])
            wos_t.append(w)

        # x^T loads behind the first slice weights on sync
        for g in range(4):
            nc.sync.dma_start(out=xt_s[:, 4 * g:4 * g + 4, :],
                              in_=xt_d[:, 4 * g * 1024:(4 * g + 4) * 1024])

        # W_vo^T[c-slice, a] = sum_b Wv[b, c-slice]^T Wo^T[b, a]
        stage = pslice.tile([P, 4, D], bf16, bufs=1)
        for ab in range(4):
            wos_ab = wos_t[ab]
            for ct in range(4):
                ps = psum.tile([P, 512], f32, tag="acc")
                for bt in range(BT):
                    nc.tensor.matmul(
                        ps[:],
                        wvs_s[:, bt, ct * P:(ct + 1) * P],
                        wos_ab[:, bt, :],
                        start=(bt == 0),
                        stop=(bt == BT - 1))
                nc.vector.tensor_copy(
                    stage[:, ct, ab * 512:(ab + 1) * 512], ps[:])
        for ab in range(4):
            for ct in range(4):
                (nc.sync if (ab + ct) % 2 == 0 else nc.scalar).dma_start(
                    out=wvo_b[ab, ct],
                    in_=stage[:, ct, ab * 512:(ab + 1) * 512])
        all_gather(wvo_b, wvo_g)
        pslice.release()

        # gathered W_vo^T quarters (a-block each) on the RIGHT side; the
        # sync ring is otherwise idle from here until the output stores,
        # so the AG-gated loads block nothing
        wvopool = tc.alloc_tile_pool(name="wvopool", bufs=1, side="right")
        wvoq = []
        for ab in range(4):
            wq_t = wvopool.tile([P, DT, 512], bf16, tag="wvoq", bufs=4)
            for r in range(GROUP):
                nc.sync.dma_start(
                    out=wq_t[:, 4 * r:4 * r + 4, :],
                    in_=wvo_g[r, ab]
                    .rearrange("ct p a -> p ct a"))
            wvoq.append(wq_t)

        # ---------------- Phase 2: q then q' = q @ Wk ----------------
        pq = tc.alloc_tile_pool(name="pq", bufs=1)
        qt_s = pq.tile([P, ET, 1024], bf16)
        for et in range(ET):
            wcol = pw.tile([P, DT, P], bf16, tag="wcol", bufs=4)
            nc.scalar.dma_start(out=wcol[:], in_=wqt_d[et])
            for ih in range(2):
                ps = psum.tile([P, 512], f32, tag="acc")
                for dt_i in range(DT):
                    nc.tensor.matmul(
                        ps[:],
                        wcol[:, dt_i, :],
                        xt_s[:, dt_i, ih * 512:(ih + 1) * 512],
                        start=(dt_i == 0),
                        stop=(dt_i == DT - 1))
                nc.vector.tensor_copy(
                    qt_s[:, et, ih * 512:(ih + 1) * 512], ps[:])

        for d2t in range(DT):
            wcol = pw.tile([P, ET, P], bf16, tag="wcol2", bufs=3)
            nc.scalar.dma_start(out=wcol[:], in_=wkt_d[d2t])
            for ih in range(2):
                ps = psum.tile([P, 512], f32, tag="acc")
                for et in range(ET):
                    nc.tensor.matmul(
                        ps[:],
                        wcol[:, et, :],
                        qt_s[:, et, ih * 512:(ih + 1) * 512],
                        start=(et == 0),
                        stop=(et == ET - 1))
                nc.vector.tensor_copy(
                    qpt[:, d2t, ih * 512:(ih + 1) * 512], ps[:])
        pq.release()
        pw.release()

        # ---------------- Phase 3: v' = x @ W_vo^T ----------------
        pv = tc.alloc_tile_pool(name="pv", bufs=2)
        for ab in range(4):
            v_st = pv.tile([P, IT, 512], bf16, tag="v_st", bufs=2)
            for jt in range(IT):
                ps = psum.tile([P, 512], f32, tag="acc")
                for ct in range(DT):
                    nc.tensor.matmul(
                        ps[:],
                        xt_s[:, ct, jt * P:(jt + 1) * P],
                        wvoq[ab][:, ct, :],
                        start=(ct == 0),
                        stop=(ct == DT - 1))
                nc.vector.tensor_copy(v_st[:, jt, :], ps[:])
            nc.gpsimd.dma_start(
                out=v_b[ab][:].rearrange("(jt p) d -> p jt d", p=P),
                in_=v_st[:])
            all_gather(v_b[ab], v_g[ab])
        pv.release()
        wvopool.release()
        pxt.release()

        # ---------------- Phase 4: scores + rowsums ----------------
        ppt = tc.alloc_tile_pool(name="ppt", bufs=1, side="right")
        pt_s = ppt.tile([P, JT, 1024], bf16)
        att = tc.alloc_tile_pool(name="att", bufs=2)
        # first ctx V-column tiles prefetch on gpsimd as soon as AG(v'0)
        # lands, long before the ctx phase. vcol slot rotation (4 tiles per
        # quarter, 6 slots) leaves each quarter's first two tiles on fresh
        # slots, so the identity consumption order never waits on a WAR.
        oct_pool = tc.alloc_tile_pool(name="oct", bufs=2, side="right")

        def vcol_load(ab):
            vcols = [oct_pool.tile([P, IT, 512], bf16, tag="vcol",
                                   bufs=6, name=f"vcol{ab}_{r}")
                     for r in range(GROUP)]
            for r in range(GROUP):
                nc.gpsimd.dma_start(
                    out=vcols[r][:],
                    in_=v_g[ab][r, :, :].rearrange("(t p) d -> p t d", p=P))
            return vcols

        vcol0 = vcol_load(0)

        for ch in range(NCH):
            xkb = att.tile([P, DT, 256], bf16, tag="xkb", bufs=3)
            eng = nc.sync if ch % 2 == 0 else nc.scalar
            for g in range(2):
                eng.dma_start(
                    out=xkb[:, 8 * g:8 * g + 8, :],
                    in_=xk_d[ch][:, 8 * g * 256:(8 * g + 8) * 256])
            for jl in range(2):
                jt = ch * 2 + jl
                for ib in range(2):
                    sps = psum.tile([P, 512], f32, tag="scores", bufs=4)
                    for dt_i in range(DT):
                        nc.tensor.matmul(
                            sps[:],
                            xkb[:, dt_i, jl * P:(jl + 1) * P],
                            qpt[:, dt_i, ib * 512:(ib + 1) * 512],
                            start=(dt_i == 0),
                            stop=(dt_i == DT - 1))
                    nc.scalar.activation(
                        pt_s[:, jt, ib * 512:(ib + 1) * 512],
                        sps[:],
                        mybir.ActivationFunctionType.Exp,
                        scale=SCALE)
        # row sums via broadcast ones-matmul (l replicated on all
        # partitions), then a 4KB DRAM bounce transposes l onto
        # i-partitions for the ctx evacuation scale. The bounce chain
        # hides behind the first ctx accumulation group.
        l_sb = att.tile([P, 2, 512], f32, bufs=1)
        for ib in range(2):
            lps = psum.tile([P, 512], f32, tag="scores", bufs=4)
            for jt in range(JT):
                nc.tensor.matmul(
                    lps[:], ones[:],
                    pt_s[:, jt, ib * 512:(ib + 1) * 512],
                    start=(jt == 0), stop=(jt == JT - 1))
            nc.vector.tensor_copy(l_sb[:, ib, :], lps[:])
        nc.sync.dma_start(out=l_d[:, :], in_=l_sb[0:1, :, :])
        l_t = att.tile([P, IT], f32, bufs=1)
        for it in range(IT):
            nc.sync.dma_start(out=l_t[:, it:it + 1], in_=l_d[it:it + 1, :])
        nc.vector.reciprocal(linv[:], l_t[:])
        att.release()
        pq2.release()

        # ---------------- Phase 5: ctx = P^T^T @ v' = output ----------
        for ab in range(4):
            vcols = vcol0 if ab == 0 else vcol_load(ab)
            for h in range(2):
                osb = oct_pool.tile([P, 4, 512], f32, tag="osb", bufs=2)
                for il in range(4):
                    it = h * 4 + il
                    cps = psum.tile([P, 512], f32, tag="scores", bufs=4)
                    for jt in range(JT):
                        nc.tensor.matmul(
                            cps[:],
                            pt_s[:, jt, it * P:(it + 1) * P],
                            vcols[jt // IT][:, jt % IT, :],
                            start=(jt == 0),
                            stop=(jt == JT - 1))
                    nc.scalar.activation(
                        osb[:, il, :], cps[:],
                        mybir.ActivationFunctionType.Copy,
                        scale=linv[:, it:it + 1])
                # merged stores; final block split so the post-last-matmul
                # tail is a short store
                last = ab == 3 and h == 1
                bounds = ((0, 4),) if not last else ((0, 2), (2, 3), (3, 4))
                for lo, hi in bounds:
                    nc.sync.dma_start(
                        out=out_d[(4 * h + lo) * P:(4 * h + hi) * P,
                                  ab * 512:(ab + 1) * 512]
                        .rearrange("(it p) f -> p it f", p=P),
                        in_=osb[:, lo:hi, :])
        oct_pool.release()
        ppt.release()
        persist.release()
        psum.release()
        dram.release()

    nc.compile()
    return nc


def _get_nc():
    if "nc" not in _CACHE:
        _CACHE["nc"] = _build()
    return _CACHE["nc"]


def _tile_we(w):
    # lhsT matrix L = w.T tiled as [mt, p, pt*128] contiguous
    wt = np.ascontiguousarray(np.asarray(w, np.float32).T)
    t = wt.reshape(DT, P, DT, P).transpose(2, 1, 0, 3)
    return np.ascontiguousarray(t.reshape(DT, P, DT * P)).astype(BF16)


def _in_maps(x, wq, wk, wv, wo):
    x = np.asarray(x, np.float32)
    wq = np.asarray(wq, np.float32)
    wk = np.asarray(wk, np.float32)
    wv = np.asarray(wv, np.float32)
    wo = np.asarray(wo, np.float32)

    wqt = _tile_we(wq)            # q-proj lhsT: Wq^T tiles [et, p=d, dt, e]
    wkt = _tile_we(wk.T)          # q'-proj lhsT: Wk tiles [d2t, p=e, et, d2]
    # rhs Wo^T[b, a] tiled [ab, p=b, bt, 512]
    wot = wo.T.reshape(BT, P, 4, 512).transpose(2, 1, 0, 3)
    wot = np.ascontiguousarray(wot.reshape(4, P, BT * 512)).astype(BF16)

    maps = []
    for c in range(NCORES):
        b, r = c // GROUP, c % GROUP
        xb = x[b]                                     # [4096, 2048]
        xl = xb[r * ROWS:(r + 1) * ROWS, :]           # [1024, 2048]
        xt = xl.T.reshape(DT, P, ROWS).transpose(1, 0, 2)
        xt = np.ascontiguousarray(xt.reshape(P, DT * ROWS)).astype(BF16)
        # x_full^T chunks [ch, p=d, dt, 256]
        xk = xb.T.reshape(DT, P, NCH, 256).transpose(2, 1, 0, 3)
        xk = np.ascontiguousarray(xk.reshape(NCH, P, DT * 256)).astype(BF16)
        # slice lhsT: Wv[:, 512-col slice] tiled [p=b, bt, 512]
        wvs = wv[:, r * 512:(r + 1) * 512].reshape(BT, P, 512)
        wvs = np.ascontiguousarray(
            wvs.transpose(1, 0, 2).reshape(P, BT * 512)).astype(BF16)
        maps.append({"xt": xt, "xk": xk, "wqt": wqt, "wkt": wkt,
                     "wvs": wvs, "wos": wot})
    return maps


def run(x, wq, wk, wv, wo, trace=False, **trace_kwargs):
    from concourse.bass_utils import run_bass_kernel_spmd
    nc = _get_nc()
    res = run_bass_kernel_spmd(nc, _in_maps(x, wq, wk, wv, wo),
                               list(range(NCORES)), trace=trace,
                               **trace_kwargs)
    out = np.empty((BATCH, S, D), np.float32)
    for c in range(NCORES):
        b, r = c // GROUP, c % GROUP
        out[b, r * ROWS:(r + 1) * ROWS, :] = res.results[c]["out"]
    return out, res


def kernel(x, wq, wk, wv, wo):
    out, _ = run(x, wq, wk, wv, wo)
    return out


# revision 23
# speedup vs baseline: 1.0180x; 1.0118x over previous
"""Distributed Bass attention kernel for 8 TRN2 NeuronCores.

Problem: full-dim attention (no head split), x:(2,4096,2048), 4x 2048^2 weights.

Sharding: batch+sequence parallel. Core c owns batch b=c//4 and query rows
[1024*(c%4), 1024*(c%4+1)).

Algebraic restructure vs the classic q/k/v/o pipeline (all bf16; fp8 was
measured numerically unusable here -- heavy-tailed scores make sharp softmax
rows pass element-level e4m3 noise straight to the output):

- chained QK: scores = ((x Wq^T) Wk) x_full^T. The second projection
  q' = q @ Wk replaces the k-projection; x_full^T is fed from the host, so
  there is NO AllGather(K) at all and the score phase has no collective
  dependency.
- fused VO: W_vo^T = Wv^T Wo^T is computed on-chip, 4-way sharded within
  each replica group (each core computes 512 rows; one AllGather within the
  group, fully hidden behind the q/q' projections). Then v' = x @ W_vo^T is
  gathered (AG per 512-col quarter, hidden behind v'-proj+scores) and
  ctx = softmax(scores) @ v' IS the final output -- no output projection.
  Net: -512 weight-column matmuls +256 slice matmuls ~= -67us of PE rows.

Schedule notes:
- Scores are computed TRANSPOSED (P^T[j,i] via lhsT=x_full^T chunk) so exp
  writes P^T directly; no max subtraction (validated numerically, rel err
  ~5e-3 vs 2e-2 budget).
- ctx is computed UNtransposed (out[i-part, a] via lhsT=P^T tiles), so the
  store to out[i, a] needs no transpose; 1/l is applied during the PSUM
  evacuation as a per-partition activation scale.
- Row sums l[i] need i on partitions -> 256 tiny FD-1 matmuls
  (lhsT = P^T tile, rhs = ones column), interleaved per score chunk so no
  serial rowsum stage exists.
- The PE is GPIO-power-throttled to 13/16 rate (~1.95 rows/ns) for ~95% of
  the kernel; wall time ~= matmul rows x 0.52ns + exposed latency. The
  schedule keeps AG(Wvo) behind the q/q' projections and AG(v') quarters
  behind v'-proj+scores so no collective is on the PE's critical path.
- DMA rings: scalar = weight-column streams then the x_full^T score
  chunks; sync = slice weights, x^T, gathered-Wvo quarters, output stores;
  gpsimd = v' bounce stores and ctx V-column loads.
"""

import numpy as np
import ml_dtypes

BF16 = ml_dtypes.bfloat16

D = 2048          # model dim
S = 4096          # sequence length per batch
BATCH = 2
NCORES = 8
GROUP = 4         # replica group size (cores per batch)
ROWS = S // GROUP  # query rows per core = 1024
P = 128           # partitions
DT = D // P       # 16 d-tiles
ET = DT
BT = DT
IT = ROWS // P    # 8 i-tiles per core
JT = S // P       # 32 j-tiles (full seq)
NCH = 16          # score chunks of 256 keys
SCALE = 1.0 / float(np.sqrt(D))

_CACHE = {}


def _build():
    from concourse import bacc, mybir, tile

    f32 = mybir.dt.float32
    bf16 = mybir.dt.bfloat16

    nc = bacc.Bacc("TRN2", target_bir_lowering=False, debug=False,
                   num_devices=NCORES)

    # host-pre-tiled inputs (see _in_maps): every load is contiguous rows
    xt_d = nc.dram_tensor("xt", [P, DT * 1024], bf16, kind="ExternalInput")
    xk_d = nc.dram_tensor("xk", [NCH, P, DT * 256], bf16,
                          kind="ExternalInput")
    wqt_d = nc.dram_tensor("wqt", [ET, P, DT * P], bf16, kind="ExternalInput")
    wkt_d = nc.dram_tensor("wkt", [DT, P, ET * P], bf16, kind="ExternalInput")
    wvs_d = nc.dram_tensor("wvs", [P, BT * 512], bf16, kind="ExternalInput")
    wos_d = nc.dram_tensor("wos", [4, P, BT * 512], bf16,
                           kind="ExternalInput")
    out_d = nc.dram_tensor("out", [ROWS, D], f32, kind="ExternalOutput")

    RG4 = [[0, 1, 2, 3], [4, 5, 6, 7]]

    def all_gather(src, dst):
        return nc.gpsimd.collective_compute(
            "AllGather", mybir.AluOpType.bypass, replica_groups=RG4,
            ins=[src.opt()], outs=[dst.opt()])

    with tile.TileContext(nc) as tc:
        dram = tc.alloc_tile_pool(name="dram", bufs=1, space="DRAM")
        persist = tc.alloc_tile_pool(name="persist", bufs=1)
        psum = tc.alloc_tile_pool(name="psum", bufs=2, space="PSUM")

        # W_vo^T slice bounce + gather; v' bounce/gather per 512-col quarter
        wvo_b = dram.tile([4, 4, P, 512], bf16, name="wvo_b")
        wvo_g = dram.tile([GROUP, 4, 4, P, 512], bf16,
                          name="wvo_g")
        v_b = [dram.tile([ROWS, 512], bf16, name=f"v_b{h}") for h in range(4)]
        v_g = [dram.tile([GROUP, ROWS, 512], bf16, name=f"v_g{h}")
               for h in range(4)]
        l_d = dram.tile([IT, P], f32, name="l_d")

        ones = persist.tile([P, P], bf16)
        linv = persist.tile([P, IT], f32)  # 1/l, i on partitions

        # q'^T: written in phase 2, read through the score phase; sits at
        # the bottom of the left stack so the LIFO release order works out
        pq2 = tc.alloc_tile_pool(name="pq2", bufs=1)
        qpt = pq2.tile([P, DT, 1024], bf16)

        # x^T: needed by q-proj (~t=75) and v'-proj; own pool, lives
        # through v'-proj
        pxt = tc.alloc_tile_pool(name="pxt", bufs=1)
        xt_s = pxt.tile([P, DT, 1024], bf16)

        # score-chunk stream pool on the RIGHT, allocated first: its
        # addresses never overlap the projection working set, so chunk
        # prefetch is not WAR-gated on v'-proj completion
        xkpool = tc.alloc_tile_pool(name="xkpool", bufs=1, side="right")

        # weight-column stream pool sits BELOW pslice so the q-proj
        # columns preload during the slice phase with no WAR gate
        pw = tc.alloc_tile_pool(name="pw", bufs=2)

        # ---------------- Phase 1: W_vo^T slice ----------------
        pslice = tc.alloc_tile_pool(name="pslice", bufs=2)

        # warm both HWDGE rings so the first real loads skip spin-up
        warm = pslice.tile([P, 16], bf16, bufs=1)
        nc.sync.dma_start(out=warm[0:1, :], in_=xt_d[0:1, 0:16])
        nc.scalar.dma_start(out=warm[1:2, :], in_=xt_d[1:2, 0:16])
        nc.gpsimd.memset(ones[:], 1.0)

        # slice lhsT (Wv columns for this core's 512 W_vo rows) in 3 chunks,
        # then the rhs quarters stream; first matmul gates on ~1.5MB
        wvs_s = pslice.tile([P, BT, 512], bf16, bufs=1)
        nc.scalar.dma_start(out=wvs_s[:, 0:4, :], in_=wvs_d[:, :4 * 512])
        nc.scalar.dma_start(out=wvs_s[:, 4:8, :],
                            in_=wvs_d[:, 4 * 512:8 * 512])
        nc.scalar.dma_start(out=wvs_s[:, 8:16, :], in_=wvs_d[:, 8 * 512:])
        wos_t = []
        for ab in range(4):
            w = pslice.tile([P, BT, 512], bf16, tag="wos", bufs=2)
            eng = nc.sync if ab % 2 == 0 else nc.scalar
            for g in range(4):
                eng.dma_start(out=w[:, 4 * g:4 * g + 4, :],
                              in_=wos_d[ab][:, 4 * g * 512:(4 * g + 4) * 512])
            wos_t.append(w)

        # x^T loads behind the first slice weights on sync
        for g in range(4):
            nc.sync.dma_start(out=xt_s[:, 4 * g:4 * g + 4, :],
                              in_=xt_d[:, 4 * g * 1024:(4 * g + 4) * 1024])

        # W_vo^T[c-slice, a] = sum_b Wv[b, c-slice]^T Wo^T[b, a]
        stage = pslice.tile([P, 4, D], bf16, bufs=1)
        for ab in range(4):
            wos_ab = wos_t[ab]
            for ct in range(4):
                ps = psum.tile([P, 512], f32, tag="acc")
                for bt in range(BT):
                    nc.tensor.matmul(
                        ps[:],
                        wvs_s[:, bt, ct * P:(ct + 1) * P],
                        wos_ab[:, bt, :],
                        start=(bt == 0),
                        stop=(bt == BT - 1))
                nc.vector.tensor_copy(
                    stage[:, ct, ab * 512:(ab + 1) * 512], ps[:])
        for ab in range(4):
            for ct in range(4):
                (nc.sync if (ab + ct) % 2 == 0 else nc.scalar).dma_start(
                    out=wvo_b[ab, ct],
                    in_=stage[:, ct, ab * 512:(ab + 1) * 512])
        all_gather(wvo_b, wvo_g)
        pslice.release()

        # gathered W_vo^T quarters (a-block each) on the RIGHT side; loads
        # ride the gpsimd ring, which is idle until the v' bounce stores,
        # so their AG-wait blocks nothing. bufs=3: quarter 3 reuses
        # quarter 0's slot, whose readers finish a full quarter earlier.
        wvopool = tc.alloc_tile_pool(name="wvopool", bufs=1, side="right")
        wvoq = []
        for ab in range(4):
            wq_t = wvopool.tile([P, DT, 512], bf16, tag="wvoq", bufs=3)
            for r in range(GROUP):
                nc.gpsimd.dma_start(
                    out=wq_t[:, 4 * r:4 * r + 4, :],
                    in_=wvo_g[r, ab]
                    .rearrange("ct p a -> p ct a"))
            wvoq.append(wq_t)

        # ---------------- Phase 2: q then q' = q @ Wk ----------------
        pq = tc.alloc_tile_pool(name="pq", bufs=1)
        qt_s = pq.tile([P, ET, 1024], bf16)
        for et in range(ET):
            wcol = pw.tile([P, DT, P], bf16, tag="wcol", bufs=4)
            nc.scalar.dma_start(out=wcol[:], in_=wqt_d[et])
            for ih in range(2):
                ps = psum.tile([P, 512], f32, tag="acc")
                for dt_i in range(DT):
                    nc.tensor.matmul(
                        ps[:],
                        wcol[:, dt_i, :],
                        xt_s[:, dt_i, ih * 512:(ih + 1) * 512],
                        start=(dt_i == 0),
                        stop=(dt_i == DT - 1))
                nc.vector.tensor_copy(
                    qt_s[:, et, ih * 512:(ih + 1) * 512], ps[:])

        for d2t in range(DT):
            wcol = pw.tile([P, ET, P], bf16, tag="wcol2", bufs=3)
            nc.sync.dma_start(out=wcol[:], in_=wkt_d[d2t])
            for ih in range(2):
                ps = psum.tile([P, 512], f32, tag="acc")
                for et in range(ET):
                    nc.tensor.matmul(
                        ps[:],
                        wcol[:, et, :],
                        qt_s[:, et, ih * 512:(ih + 1) * 512],
                        start=(et == 0),
                        stop=(et == ET - 1))
                nc.vector.tensor_copy(
                    qpt[:, d2t, ih * 512:(ih + 1) * 512], ps[:])
        pq.release()
        pw.release()

        # ---------------- Phase 3: v' = x @ W_vo^T ----------------
        pv = tc.alloc_tile_pool(name="pv", bufs=2)
        for ab in range(4):
            v_st = pv.tile([P, IT, 512], bf16, tag="v_st", bufs=2)
            for jt in range(IT):
                ps = psum.tile([P, 512], f32, tag="acc")
                for ct in range(DT):
                    nc.tensor.matmul(
                        ps[:],
                        xt_s[:, ct, jt * P:(jt + 1) * P],
                        wvoq[ab][:, ct, :],
                        start=(ct == 0),
                        stop=(ct == DT - 1))
                nc.vector.tensor_copy(v_st[:, jt, :], ps[:])
            nc.gpsimd.dma_start(
                out=v_b[ab][:].rearrange("(jt p) d -> p jt d", p=P),
                in_=v_st[:])
            all_gather(v_b[ab], v_g[ab])
        pv.release()
        wvopool.release()
        pxt.release()

        # ---------------- Phase 4: scores + rowsums ----------------
        ppt = tc.alloc_tile_pool(name="ppt", bufs=1, side="right")
        pt_s = ppt.tile([P, JT, 1024], bf16)
        att = tc.alloc_tile_pool(name="att", bufs=2)
        # first ctx V-column tiles prefetch on gpsimd as soon as AG(v'0)
        # lands, long before the ctx phase. vcol slot rotation (4 tiles per
        # quarter, 6 slots) leaves each quarter's first two tiles on fresh
        # slots, so the identity consumption order never waits on a WAR.
        oct_pool = tc.alloc_tile_pool(name="oct", bufs=2, side="right")

        def vcol_load(ab):
            vcols = [oct_pool.tile([P, IT, 512], bf16, tag="vcol",
                                   bufs=6, name=f"vcol{ab}_{r}")
                     for r in range(GROUP)]
            for r in range(GROUP):
                nc.gpsimd.dma_start(
                    out=vcols[r][:],
                    in_=v_g[ab][r, :, :].rearrange("(t p) d -> p t d", p=P))
            return vcols

        vcol0 = vcol_load(0)

        for ch in range(NCH):
            xkb = xkpool.tile([P, DT, 256], bf16, tag="xkb", bufs=2)
            eng = nc.sync if ch % 2 == 0 else nc.scalar
            for g in range(2):
                eng.dma_start(
                    out=xkb[:, 8 * g:8 * g + 8, :],
                    in_=xk_d[ch][:, 8 * g * 256:(8 * g + 8) * 256])
            for jl in range(2):
                jt = ch * 2 + jl
                for ib in range(2):
                    sps = psum.tile([P, 512], f32, tag="scores", bufs=4)
                    for dt_i in range(DT):
                        nc.tensor.matmul(
                            sps[:],
                            xkb[:, dt_i, jl * P:(jl + 1) * P],
                            qpt[:, dt_i, ib * 512:(ib + 1) * 512],
                            start=(dt_i == 0),
                            stop=(dt_i == DT - 1))
                    nc.scalar.activation(
                        pt_s[:, jt, ib * 512:(ib + 1) * 512],
                        sps[:],
                        mybir.ActivationFunctionType.Exp,
                        scale=SCALE)
        # row sums via broadcast ones-matmul (l replicated on all
        # partitions), then a 4KB DRAM bounce transposes l onto
        # i-partitions for the ctx evacuation scale. The bounce chain
        # hides behind the first ctx accumulation group.
        l_sb = att.tile([P, 2, 512], f32, bufs=1)
        for ib in range(2):
            lps = psum.tile([P, 512], f32, tag="scores", bufs=4)
            for jt in range(JT):
                nc.tensor.matmul(
                    lps[:], ones[:],
                    pt_s[:, jt, ib * 512:(ib + 1) * 512],
                    start=(jt == 0), stop=(jt == JT - 1))
            nc.vector.tensor_copy(l_sb[:, ib, :], lps[:])
        nc.sync.dma_start(out=l_d[:, :], in_=l_sb[0:1, :, :])
        l_t = att.tile([P, IT], f32, bufs=1)
        for it in range(IT):
            nc.sync.dma_start(out=l_t[:, it:it + 1], in_=l_d[it:it + 1, :])
        nc.vector.reciprocal(linv[:], l_t[:])
        att.release()
        pq2.release()

        # ---------------- Phase 5: ctx = P^T^T @ v' = output ----------
        for ab in range(4):
            vcols = vcol0 if ab == 0 else vcol_load(ab)
            for h in range(2):
                osb = oct_pool.tile([P, 4, 512], f32, tag="osb", bufs=2)
                for il in range(4):
                    it = h * 4 + il
                    cps = psum.tile([P, 512], f32, tag="scores", bufs=4)
                    for jt in range(JT):
                        nc.tensor.matmul(
                            cps[:],
                            pt_s[:, jt, it * P:(it + 1) * P],
                            vcols[jt // IT][:, jt % IT, :],
                            start=(jt == 0),
                            stop=(jt == JT - 1))
                    nc.scalar.activation(
                        osb[:, il, :], cps[:],
                        mybir.ActivationFunctionType.Copy,
                        scale=linv[:, it:it + 1])
                # merged stores; final block split so the post-last-matmul
                # tail is a short store
                last = ab == 3 and h == 1
                bounds = ((0, 4),) if not last else ((0, 2), (2, 3), (3, 4))
                for lo, hi in bounds:
                    nc.sync.dma_start(
                        out=out_d[(4 * h + lo) * P:(4 * h + hi) * P,
                                  ab * 512:(ab + 1) * 512]
                        .rearrange("(it p) f -> p it f", p=P),
                        in_=osb[:, lo:hi, :])
        oct_pool.release()
        ppt.release()
        xkpool.release()
        persist.release()
        psum.release()
        dram.release()

    nc.compile()
    return nc


def _get_nc():
    if "nc" not in _CACHE:
        _CACHE["nc"] = _build()
    return _CACHE["nc"]


def _tile_we(w):
    # lhsT matrix L = w.T tiled as [mt, p, pt*128] contiguous
    wt = np.ascontiguousarray(np.asarray(w, np.float32).T)
    t = wt.reshape(DT, P, DT, P).transpose(2, 1, 0, 3)
    return np.ascontiguousarray(t.reshape(DT, P, DT * P)).astype(BF16)


def _in_maps(x, wq, wk, wv, wo):
    x = np.asarray(x, np.float32)
    wq = np.asarray(wq, np.float32)
    wk = np.asarray(wk, np.float32)
    wv = np.asarray(wv, np.float32)
    wo = np.asarray(wo, np.float32)

    wqt = _tile_we(wq)            # q-proj lhsT: Wq^T tiles [et, p=d, dt, e]
    wkt = _tile_we(wk.T)          # q'-proj lhsT: Wk tiles [d2t, p=e, et, d2]
    # rhs Wo^T[b, a] tiled [ab, p=b, bt, 512]
    wot = wo.T.reshape(BT, P, 4, 512).transpose(2, 1, 0, 3)
    wot = np.ascontiguousarray(wot.reshape(4, P, BT * 512)).astype(BF16)

    maps = []
    for c in range(NCORES):
        b, r = c // GROUP, c % GROUP
        xb = x[b]                                     # [4096, 2048]
        xl = xb[r * ROWS:(r + 1) * ROWS, :]           # [1024, 2048]
        xt = xl.T.reshape(DT, P, ROWS).transpose(1, 0, 2)
        xt = np.ascontiguousarray(xt.reshape(P, DT * ROWS)).astype(BF16)
        # x_full^T chunks [ch, p=d, dt, 256]
        xk = xb.T.reshape(DT, P, NCH, 256).transpose(2, 1, 0, 3)
        xk = np.ascontiguousarray(xk.reshape(NCH, P, DT * 256)).astype(BF16)
        # slice lhsT: Wv[:, 512-col slice] tiled [p=b, bt, 512]
        wvs = wv[:, r * 512:(r + 1) * 512].reshape(BT, P, 512)
        wvs = np.ascontiguousarray(
            wvs.transpose(1, 0, 2).reshape(P, BT * 512)).astype(BF16)
        maps.append({"xt": xt, "xk": xk, "wqt": wqt, "wkt": wkt,
                     "wvs": wvs, "wos": wot})
    return maps


def run(x, wq, wk, wv, wo, trace=False, **trace_kwargs):
    from concourse.bass_utils import run_bass_kernel_spmd
    nc = _get_nc()
    res = run_bass_kernel_spmd(nc, _in_maps(x, wq, wk, wv, wo),
                               list(range(NCORES)), trace=trace,
                               **trace_kwargs)
    out = np.empty((BATCH, S, D), np.float32)
    for c in range(NCORES):
        b, r = c // GROUP, c % GROUP
        out[b, r * ROWS:(r + 1) * ROWS, :] = res.results[c]["out"]
    return out, res


def kernel(x, wq, wk, wv, wo):
    out, _ = run(x, wq, wk, wv, wo)
    return out


# revision 25
# speedup vs baseline: 1.0640x; 1.0452x over previous
"""Distributed Bass attention kernel for 8 TRN2 NeuronCores.

Problem: full-dim attention (no head split), x:(2,4096,2048), 4x 2048^2 weights.

Sharding: batch+sequence parallel. Core c owns batch b=c//4 and query rows
[1024*(c%4), 1024*(c%4+1)).

Algebraic restructure vs the classic q/k/v/o pipeline (all bf16; fp8 was
measured numerically unusable here -- heavy-tailed scores make sharp softmax
rows pass element-level e4m3 noise straight to the output):

- chained QK: scores = ((x Wq^T) Wk) x_full^T. The second projection
  q' = q @ Wk replaces the k-projection; x_full^T is fed from the host, so
  there is NO AllGather(K) at all and the score phase has no collective
  dependency.
- fused VO: W_vo^T = Wv^T Wo^T is computed on-chip, 4-way sharded within
  each replica group (each core computes 512 rows; one AllGather within the
  group, fully hidden behind the q/q' projections). Then v' = x @ W_vo^T is
  gathered (AG per 512-col quarter, hidden behind v'-proj+scores) and
  ctx = softmax(scores) @ v' IS the final output -- no output projection.
  Net: -512 weight-column matmuls +256 slice matmuls ~= -67us of PE rows.

Schedule notes:
- Scores are computed TRANSPOSED (P^T[j,i] via lhsT=x_full^T chunk) so exp
  writes P^T directly; no max subtraction (validated numerically, rel err
  ~5e-3 vs 2e-2 budget).
- ctx is computed UNtransposed (out[i-part, a] via lhsT=P^T tiles), so the
  store to out[i, a] needs no transpose; 1/l is applied during the PSUM
  evacuation as a per-partition activation scale.
- Row sums l[i] need i on partitions -> 256 tiny FD-1 matmuls
  (lhsT = P^T tile, rhs = ones column), interleaved per score chunk so no
  serial rowsum stage exists.
- The PE is GPIO-power-throttled to 13/16 rate (~1.95 rows/ns) for ~95% of
  the kernel; wall time ~= matmul rows x 0.52ns + exposed latency. The
  schedule keeps AG(Wvo) behind the q/q' projections and AG(v') quarters
  behind v'-proj+scores so no collective is on the PE's critical path.
- DMA rings: scalar = weight-column streams then the x_full^T score
  chunks; sync = slice weights, x^T, gathered-Wvo quarters, output stores;
  gpsimd = v' bounce stores and ctx V-column loads.
"""

import numpy as np
import ml_dtypes

BF16 = ml_dtypes.bfloat16

D = 2048          # model dim
S = 4096          # sequence length per batch
BATCH = 2
NCORES = 8
GROUP = 4         # replica group size (cores per batch)
ROWS = S // GROUP  # query rows per core = 1024
P = 128           # partitions
DT = D // P       # 16 d-tiles
ET = DT
BT = DT
IT = ROWS // P    # 8 i-tiles per core
JT = S // P       # 32 j-tiles (full seq)
NCH = 16          # score chunks of 256 keys
SCALE = 1.0 / float(np.sqrt(D))

_CACHE = {}


def _build():
    from concourse import bacc, mybir, tile

    f32 = mybir.dt.float32
    bf16 = mybir.dt.bfloat16

    nc = bacc.Bacc("TRN2", target_bir_lowering=False, debug=False,
                   num_devices=NCORES)

    # host-pre-tiled inputs (see _in_maps): every load is contiguous rows
    xt_d = nc.dram_tensor("xt", [P, DT * 1024], bf16, kind="ExternalInput")
    xk_d = nc.dram_tensor("xk", [JT, P, DT * P], bf16,
                          kind="ExternalInput")
    wqt_d = nc.dram_tensor("wqt", [ET, P, DT * P], bf16, kind="ExternalInput")
    wkt_d = nc.dram_tensor("wkt", [DT, P, ET * P], bf16, kind="ExternalInput")
    wvs_d = nc.dram_tensor("wvs", [P, BT * 512], bf16, kind="ExternalInput")
    wos_d = nc.dram_tensor("wos", [4, P, BT * 512], bf16,
                           kind="ExternalInput")
    out_d = nc.dram_tensor("out", [ROWS, D], f32, kind="ExternalOutput")

    RG4 = [[0, 1, 2, 3], [4, 5, 6, 7]]

    def all_gather(src, dst):
        return nc.gpsimd.collective_compute(
            "AllGather", mybir.AluOpType.bypass, replica_groups=RG4,
            ins=[src.opt()], outs=[dst.opt()])

    with tile.TileContext(nc) as tc:
        dram = tc.alloc_tile_pool(name="dram", bufs=1, space="DRAM")
        persist = tc.alloc_tile_pool(name="persist", bufs=1)
        psum = tc.alloc_tile_pool(name="psum", bufs=2, space="PSUM")

        # W_vo^T slice bounce + gather; v' bounce/gather per 512-col quarter
        wvo_b = dram.tile([4, 4, P, 512], bf16, name="wvo_b")
        wvo_g = dram.tile([GROUP, 4, 4, P, 512], bf16,
                          name="wvo_g")
        v_b = [dram.tile([ROWS, 512], bf16, name=f"v_b{h}") for h in range(4)]
        v_g = [dram.tile([GROUP, ROWS, 512], bf16, name=f"v_g{h}")
               for h in range(4)]
        l_d = dram.tile([IT, P], f32, name="l_d")

        ones = persist.tile([P, P], bf16)
        linv = persist.tile([P, IT], f32)  # 1/l, i on partitions

        # q'^T: written in phase 2, read through the score phase; sits at
        # the bottom of the left stack so the LIFO release order works out
        pq2 = tc.alloc_tile_pool(name="pq2", bufs=1)
        qpt = pq2.tile([P, DT, 1024], bf16)

        # x^T: needed by q-proj (~t=75) and v'-proj; own pool, lives
        # through v'-proj
        pxt = tc.alloc_tile_pool(name="pxt", bufs=1)
        xt_s = pxt.tile([P, DT, 1024], bf16)

        # score-chunk stream pool on the RIGHT, allocated first: its
        # addresses never overlap the projection working set, so chunk
        # prefetch is not WAR-gated on v'-proj completion
        xkpool = tc.alloc_tile_pool(name="xkpool", bufs=1, side="right")

        # weight-column stream pool sits BELOW pslice so the q-proj
        # columns preload during the slice phase with no WAR gate
        pw = tc.alloc_tile_pool(name="pw", bufs=2)

        # ---------------- Phase 1: W_vo^T slice ----------------
        pslice = tc.alloc_tile_pool(name="pslice", bufs=2)

        # warm both HWDGE rings so the first real loads skip spin-up
        warm = pslice.tile([P, 16], bf16, bufs=1)
        nc.sync.dma_start(out=warm[0:1, :], in_=xt_d[0:1, 0:16])
        nc.scalar.dma_start(out=warm[1:2, :], in_=xt_d[1:2, 0:16])
        nc.gpsimd.memset(ones[:], 1.0)

        # slice lhsT (Wv columns for this core's 512 W_vo rows) in 3 chunks,
        # then the rhs quarters stream; first matmul gates on ~1.5MB
        wvs_s = pslice.tile([P, BT, 512], bf16, bufs=1)
        nc.scalar.dma_start(out=wvs_s[:, 0:4, :], in_=wvs_d[:, :4 * 512])
        nc.scalar.dma_start(out=wvs_s[:, 4:8, :],
                            in_=wvs_d[:, 4 * 512:8 * 512])
        nc.scalar.dma_start(out=wvs_s[:, 8:16, :], in_=wvs_d[:, 8 * 512:])
        wos_t = []
        for ab in range(4):
            w = pslice.tile([P, BT, 512], bf16, tag="wos", bufs=2)
            eng = nc.sync if ab % 2 == 0 else nc.scalar
            for g in range(4):
                eng.dma_start(out=w[:, 4 * g:4 * g + 4, :],
                              in_=wos_d[ab][:, 4 * g * 512:(4 * g + 4) * 512])
            wos_t.append(w)

        # x^T loads behind the first slice weights on sync
        for g in range(4):
            nc.sync.dma_start(out=xt_s[:, 4 * g:4 * g + 4, :],
                              in_=xt_d[:, 4 * g * 1024:(4 * g + 4) * 1024])

        # W_vo^T[c-slice, a] = sum_b Wv[b, c-slice]^T Wo^T[b, a]
        stage = pslice.tile([P, 4, D], bf16, bufs=1)
        for ab in range(4):
            wos_ab = wos_t[ab]
            for ct in range(4):
                ps = psum.tile([P, 512], f32, tag="acc")
                for bt in range(BT):
                    nc.tensor.matmul(
                        ps[:],
                        wvs_s[:, bt, ct * P:(ct + 1) * P],
                        wos_ab[:, bt, :],
                        start=(bt == 0),
                        stop=(bt == BT - 1))
                nc.vector.tensor_copy(
                    stage[:, ct, ab * 512:(ab + 1) * 512], ps[:])
        for ab in range(4):
            for ct in range(4):
                (nc.sync if (ab + ct) % 2 == 0 else nc.scalar).dma_start(
                    out=wvo_b[ab, ct],
                    in_=stage[:, ct, ab * 512:(ab + 1) * 512])
        all_gather(wvo_b, wvo_g)
        pslice.release()

        # gathered W_vo^T quarters (a-block each) on the RIGHT side; loads
        # ride the gpsimd ring, which is idle until the v' bounce stores,
        # so their AG-wait blocks nothing. bufs=3: quarter 3 reuses
        # quarter 0's slot, whose readers finish a full quarter earlier.
        wvopool = tc.alloc_tile_pool(name="wvopool", bufs=1, side="right")
        wvoq = []
        for ab in range(4):
            wq_t = wvopool.tile([P, DT, 512], bf16, tag="wvoq", bufs=3)
            for r in range(GROUP):
                nc.gpsimd.dma_start(
                    out=wq_t[:, 4 * r:4 * r + 4, :],
                    in_=wvo_g[r, ab]
                    .rearrange("ct p a -> p ct a"))
            wvoq.append(wq_t)

        # ---------------- Phase 2: q then q' = q @ Wk ----------------
        pq = tc.alloc_tile_pool(name="pq", bufs=1)
        qt_s = pq.tile([P, ET, 1024], bf16)
        for et in range(ET):
            wcol = pw.tile([P, DT, P], bf16, tag="wcol", bufs=3)
            (nc.scalar if et % 2 == 0 else nc.sync).dma_start(
                out=wcol[:], in_=wqt_d[et])
            for ih in range(2):
                ps = psum.tile([P, 512], f32, tag="acc")
                for dt_i in range(DT):
                    nc.tensor.matmul(
                        ps[:],
                        wcol[:, dt_i, :],
                        xt_s[:, dt_i, ih * 512:(ih + 1) * 512],
                        start=(dt_i == 0),
                        stop=(dt_i == DT - 1))
                nc.vector.tensor_copy(
                    qt_s[:, et, ih * 512:(ih + 1) * 512], ps[:])

        for d2t in range(DT):
            wcol = pw.tile([P, ET, P], bf16, tag="wcol2", bufs=2)
            (nc.sync if d2t % 2 == 0 else nc.scalar).dma_start(
                out=wcol[:], in_=wkt_d[d2t])
            for ih in range(2):
                ps = psum.tile([P, 512], f32, tag="acc")
                for et in range(ET):
                    nc.tensor.matmul(
                        ps[:],
                        wcol[:, et, :],
                        qt_s[:, et, ih * 512:(ih + 1) * 512],
                        start=(et == 0),
                        stop=(et == ET - 1))
                nc.vector.tensor_copy(
                    qpt[:, d2t, ih * 512:(ih + 1) * 512], ps[:])
        pq.release()
        pw.release()

        # ---------------- Phase 3: v' = x @ W_vo^T ----------------
        pv = tc.alloc_tile_pool(name="pv", bufs=2)
        for ab in range(4):
            v_st = pv.tile([P, IT, 512], bf16, tag="v_st", bufs=2)
            for jt in range(IT):
                ps = psum.tile([P, 512], f32, tag="acc")
                for ct in range(DT):
                    nc.tensor.matmul(
                        ps[:],
                        xt_s[:, ct, jt * P:(jt + 1) * P],
                        wvoq[ab][:, ct, :],
                        start=(ct == 0),
                        stop=(ct == DT - 1))
                nc.vector.tensor_copy(v_st[:, jt, :], ps[:])
            nc.gpsimd.dma_start(
                out=v_b[ab][:].rearrange("(jt p) d -> p jt d", p=P),
                in_=v_st[:])
            all_gather(v_b[ab], v_g[ab])
        pv.release()
        wvopool.release()
        pxt.release()

        # ---------------- Phase 4: scores + rowsums ----------------
        ppt = tc.alloc_tile_pool(name="ppt", bufs=1, side="right")
        pt_s = ppt.tile([P, JT, 1024], bf16)
        att = tc.alloc_tile_pool(name="att", bufs=2)
        # first ctx V-column tiles prefetch on gpsimd as soon as AG(v'0)
        # lands, long before the ctx phase. vcol slot rotation (4 tiles per
        # quarter, 6 slots) leaves each quarter's first two tiles on fresh
        # slots, so the identity consumption order never waits on a WAR.
        oct_pool = tc.alloc_tile_pool(name="oct", bufs=2, side="right")

        def vcol_load(ab):
            vcols = [oct_pool.tile([P, IT, 512], bf16, tag="vcol",
                                   bufs=6, name=f"vcol{ab}_{r}")
                     for r in range(GROUP)]
            for r in range(GROUP):
                nc.gpsimd.dma_start(
                    out=vcols[r][:],
                    in_=v_g[ab][r, :, :].rearrange("(t p) d -> p t d", p=P))
            return vcols

        vcol0 = vcol_load(0)

        for jt in range(JT):
            xkb = xkpool.tile([P, DT, P], bf16, tag="xkb", bufs=6)
            nc.sync.dma_start(out=xkb[:], in_=xk_d[jt])
            for ib in range(2):
                sps = psum.tile([P, 512], f32, tag="scores", bufs=4)
                for dt_i in range(DT):
                    nc.tensor.matmul(
                        sps[:],
                        xkb[:, dt_i, :],
                        qpt[:, dt_i, ib * 512:(ib + 1) * 512],
                        start=(dt_i == 0),
                        stop=(dt_i == DT - 1))
                nc.scalar.activation(
                    pt_s[:, jt, ib * 512:(ib + 1) * 512],
                    sps[:],
                    mybir.ActivationFunctionType.Exp,
                    scale=SCALE)
        # row sums via broadcast ones-matmul (l replicated on all
        # partitions), then a 4KB DRAM bounce transposes l onto
        # i-partitions for the ctx evacuation scale. The bounce chain
        # hides behind the first ctx accumulation group.
        l_sb = att.tile([P, 2, 512], f32, bufs=1)
        for ib in range(2):
            lps = psum.tile([P, 512], f32, tag="scores", bufs=4)
            for jt in range(JT):
                nc.tensor.matmul(
                    lps[:], ones[:],
                    pt_s[:, jt, ib * 512:(ib + 1) * 512],
                    start=(jt == 0), stop=(jt == JT - 1))
            nc.vector.tensor_copy(l_sb[:, ib, :], lps[:])
        nc.sync.dma_start(out=l_d[:, :], in_=l_sb[0:1, :, :])
        l_t = att.tile([P, IT], f32, bufs=1)
        for it in range(IT):
            nc.sync.dma_start(out=l_t[:, it:it + 1], in_=l_d[it:it + 1, :])
        nc.vector.reciprocal(linv[:], l_t[:])
        att.release()
        pq2.release()

        # ---------------- Phase 5: ctx = P^T^T @ v' = output ----------
        for ab in range(4):
            vcols = vcol0 if ab == 0 else vcol_load(ab)
            for h in range(2):
                osb = oct_pool.tile([P, 4, 512], f32, tag="osb", bufs=2)
                for il in range(4):
                    it = h * 4 + il
                    cps = psum.tile([P, 512], f32, tag="scores", bufs=4)
                    for jt in range(JT):
                        nc.tensor.matmul(
                            cps[:],
                            pt_s[:, jt, it * P:(it + 1) * P],
                            vcols[jt // IT][:, jt % IT, :],
                            start=(jt == 0),
                            stop=(jt == JT - 1))
                    nc.scalar.activation(
                        osb[:, il, :], cps[:],
                        mybir.ActivationFunctionType.Copy,
                        scale=linv[:, it:it + 1])
                # merged stores; final block split so the post-last-matmul
                # tail is a short store
                last = ab == 3 and h == 1
                bounds = ((0, 4),) if not last else ((0, 2), (2, 3), (3, 4))
                for lo, hi in bounds:
                    nc.scalar.dma_start(
                        out=out_d[(4 * h + lo) * P:(4 * h + hi) * P,
                                  ab * 512:(ab + 1) * 512]
                        .rearrange("(it p) f -> p it f", p=P),
                        in_=osb[:, lo:hi, :])
        oct_pool.release()
        ppt.release()
        xkpool.release()
        persist.release()
        psum.release()
        dram.release()

    nc.compile()
    return nc


def _get_nc():
    if "nc" not in _CACHE:
        _CACHE["nc"] = _build()
    return _CACHE["nc"]


def _tile_we(w):
    # lhsT matrix L = w.T tiled as [mt, p, pt*128] contiguous
    wt = np.ascontiguousarray(np.asarray(w, np.float32).T)
    t = wt.reshape(DT, P, DT, P).transpose(2, 1, 0, 3)
    return np.ascontiguousarray(t.reshape(DT, P, DT * P)).astype(BF16)


def _in_maps(x, wq, wk, wv, wo):
    x = np.asarray(x, np.float32)
    wq = np.asarray(wq, np.float32)
    wk = np.asarray(wk, np.float32)
    wv = np.asarray(wv, np.float32)
    wo = np.asarray(wo, np.float32)

    wqt = _tile_we(wq)            # q-proj lhsT: Wq^T tiles [et, p=d, dt, e]
    wkt = _tile_we(wk.T)          # q'-proj lhsT: Wk tiles [d2t, p=e, et, d2]
    # rhs Wo^T[b, a] tiled [ab, p=b, bt, 512]
    wot = wo.T.reshape(BT, P, 4, 512).transpose(2, 1, 0, 3)
    wot = np.ascontiguousarray(wot.reshape(4, P, BT * 512)).astype(BF16)

    maps = []
    for c in range(NCORES):
        b, r = c // GROUP, c % GROUP
        xb = x[b]                                     # [4096, 2048]
        xl = xb[r * ROWS:(r + 1) * ROWS, :]           # [1024, 2048]
        xt = xl.T.reshape(DT, P, ROWS).transpose(1, 0, 2)
        xt = np.ascontiguousarray(xt.reshape(P, DT * ROWS)).astype(BF16)
        # x_full^T half-chunks [jt, p=d, dt, 128]
        xk = xb.T.reshape(DT, P, JT, P).transpose(2, 1, 0, 3)
        xk = np.ascontiguousarray(xk.reshape(JT, P, DT * P)).astype(BF16)
        # slice lhsT: Wv[:, 512-col slice] tiled [p=b, bt, 512]
        wvs = wv[:, r * 512:(r + 1) * 512].reshape(BT, P, 512)
        wvs = np.ascontiguousarray(
            wvs.transpose(1, 0, 2).reshape(P, BT * 512)).astype(BF16)
        maps.append({"xt": xt, "xk": xk, "wqt": wqt, "wkt": wkt,
                     "wvs": wvs, "wos": wot})
    return maps


def run(x, wq, wk, wv, wo, trace=False, **trace_kwargs):
    from concourse.bass_utils import run_bass_kernel_spmd
    nc = _get_nc()
    res = run_bass_kernel_spmd(nc, _in_maps(x, wq, wk, wv, wo),
                               list(range(NCORES)), trace=trace,
                               **trace_kwargs)
    out = np.empty((BATCH, S, D), np.float32)
    for c in range(NCORES):
        b, r = c // GROUP, c % GROUP
        out[b, r * ROWS:(r + 1) * ROWS, :] = res.results[c]["out"]
    return out, res


def kernel(x, wq, wk, wv, wo):
    out, _ = run(x, wq, wk, wv, wo)
    return out
